# revision 6
# baseline (speedup 1.0000x reference)
"""Quantized AP loss (R2D2 QAPLoss) on 8 Trainium2 NeuronCores.

Sharding: data-parallel over batch (4 images) x query-pixel halves
(2 halves of 2304 pixels) = 8 shards, one per core.

Device algorithm per core (1152 query pixels i, J_PAD masked db columns j):
  DB[c,j]   = sum_t wb[t,c,j] * taps[t,c,j]          (bilinear interp, DVE)
  S[i,j]    = sum_c qT[c,i] * DB[c,j] - 50*pad[j]    (TensorE + rank-1 pad fix)
  t[i,j]    = 9.5 * S[i,j]  (fp16)                   (PSUM->SBUF copy w/ scale)
  tL[i,j]   = (t + 16) * L[i,j]                      (positive-window mask)
  R[m][i]   = sum_j relu(t + m - 9.5)   m=1..19      (relu+accum passes)
  R[20][i]  = sum_j t + r20fix                       (linear shortcut)
  Rb[m][i]  = sum_j relu(tL + m - 25.5) m=1..19
  Rb[20][i] = sum_j tL - 5.5*cntL[i]
  cum_nbs_k = R[k+1]-R[k]; cum_rec_k = Rb[k+1]-Rb[k]     (telescoped clip sums)
  ap_i      = sum_k prec_k*rec_k / rec_total ; loss_i = 0.5 - r_i*(ap_i-0.5)

The identity used: cumsum_{k'<=k} tri_{k'}(s) = clip(9.5*s + k - 8.5, 0, 1)
and clip(x+1,0,1) = relu(x+1) - relu(x), so each quantization bin costs one
relu-accumulate pass, split between the Vector and Scalar engines.
"""
import sys

if "/opt/trn_rl_repo" not in sys.path:
    sys.path.insert(0, "/opt/trn_rl_repo")

import numpy as np

B, C, H, W = 4, 128, 48, 48
HW = H * W
HALF = HW // 2          # 1152 query pixels per core
NT = HALF // 128        # 9 i-tiles per core
NQ = 20
WIN = 4
A = 9.5                 # (NQ-1)/(max-min)
BIG = 16.0              # tL offset (keeps fp16 precision, > max bin bias 10.5)
PADSCORE = 50.0         # rank-1 score pushed onto padded columns
N_CORES = 8

# engine split for the 2*(NQ-1)=38 relu-accum passes (tuned from traces)
ACT_NBS_MS = set(range(1, 11))  # nbs bins handled by ScalarE
ACT_REC_MS = set(range(1, 7))   # rec bins handled by ScalarE


def _host_prep(d1, d2, rel, grid, mask):
    """Build per-core device inputs. Pure indexing / sharding; all FLOP-bearing
    work (interpolation arithmetic, matmuls, binning) happens on device."""
    maskw = mask.reshape(B, HW) == 1
    counts = maskw.sum(1)
    J_PAD = max(128, int(np.ceil(counts.max() / 128) * 128))

    xs = np.arange(HW) // W
    ys = np.arange(HW) % W

    # per (image, i-tile) window of compact columns that can hold positives:
    # rows [xlo-4, xhi+4] of the image; contiguous in compact space.
    cumcnt = np.zeros((B, HW + 1), np.int64)
    for b in range(B):
        cumcnt[b, 1:] = np.cumsum(maskw[b])
    los = np.zeros((B, 2, NT), np.int64)
    his = np.zeros((B, 2, NT), np.int64)
    for b in range(B):
        for h in (0, 1):
            for T in range(NT):
                i0 = h * HALF + T * 128
                i1 = i0 + 127
                xlo = max(i0 // W - WIN, 0)
                xhi = min(i1 // W + WIN, H - 1)
                los[b, h, T] = cumcnt[b, xlo * W]
                his[b, h, T] = cumcnt[b, (xhi + 1) * W]
    wmax = int((his - los).max())
    W_BAND = min(max(128, int(np.ceil((wmax + 2) / 128) * 128)), J_PAD)

    biases = np.zeros((128, 2 * (NQ + 1)), np.float32)
    for m in range(NQ + 1):
        biases[:, m] = m - A
        biases[:, NQ + 1 + m] = m - A - BIG

    in_maps = []
    for b in range(B):
        g = grid[b]
        gx = (g[..., 0] + 1.0) * W / 2.0 - 0.5
        gy = (g[..., 1] + 1.0) * H / 2.0 - 0.5
        x0 = np.floor(gx)
        y0 = np.floor(gy)
        wx1 = gx - x0
        wx0 = 1.0 - wx1
        wy1 = gy - y0
        wy0 = 1.0 - wy1

        jsel = np.nonzero(maskw[b])[0]
        J_valid = len(jsel)
        n_pad = J_PAD - J_valid
        d2flat = d2[b].reshape(C, HW)

        taps = np.zeros((4, C, J_PAD), np.float16)
        wb = np.zeros((4, C, J_PAD), np.float16)
        for t, (xi, yi, wv) in enumerate(
            ((x0, y0, wx0 * wy0), (x0 + 1, y0, wx1 * wy0),
             (x0, y0 + 1, wx0 * wy1), (x0 + 1, y0 + 1, wx1 * wy1))):
            valid = (xi >= 0) & (xi < W) & (yi >= 0) & (yi < H)
            xc = np.clip(xi, 0, W - 1).astype(np.int64)
            yc = np.clip(yi, 0, H - 1).astype(np.int64)
            f = (yc * W + xc).reshape(HW)[jsel]
            wt = (wv * valid).reshape(HW)[jsel]
            taps[t, :, :J_valid] = d2flat[:, f].astype(np.float16)
            wb[t, :, :J_valid] = wt.astype(np.float16)[None, :]

        padind = np.zeros((1, J_PAD), np.float16)
        padind[0, J_valid:] = 1.0
        r20fix = np.full((128, 1),
                         (A + 1.0) * J_PAD + (PADSCORE * A - (A + 1.0)) * n_pad,
                         np.float32)

        xs_j = xs[jsel]
        ys_j = ys[jsel]
        for h in (0, 1):
            irange = np.arange(h * HALF, (h + 1) * HALF)
            L = ((np.abs(xs[irange][:, None] - xs_j[None, :]) <= WIN)
                 & (np.abs(ys[irange][:, None] - ys_j[None, :]) <= WIN))
            Lp = np.zeros((HALF, J_PAD), np.float16)
            Lp[:, :J_valid] = L
            # band-aligned positive mask: L_band[T*128+p, :] =
            #   Lp[T*128+p, cstart[T]:cstart[T]+W_BAND]
            cstarts = np.zeros((1, NT), np.int32)
            L_band = np.zeros((HALF, W_BAND), np.float16)
            for T in range(NT):
                cs = int(min(max(los[b, h, T] - 1, 0), J_PAD - W_BAND))
                cs &= ~1  # even start keeps fp16 slices 4B-aligned
                cstarts[0, T] = cs
                L_band[T * 128:(T + 1) * 128] = Lp[T * 128:(T + 1) * 128,
                                                   cs:cs + W_BAND]
            assert int((L_band.sum())) == int(Lp.sum())
            cntL = np.ascontiguousarray(
                L.sum(1).astype(np.float32).reshape(NT, 128).T)
            relc = np.ascontiguousarray(
                rel[b, 0].reshape(HW)[irange].astype(np.float32)
                .reshape(NT, 128).T)
            qT = np.ascontiguousarray(
                d1[b].reshape(C, HW)[:, irange].astype(np.float16))
            in_maps.append({
                "taps": taps, "wb": wb, "qT": qT, "L": L_band,
                "cstarts": cstarts,
                "padind": padind, "r20fix": r20fix, "cntL": cntL,
                "relc": relc, "biases": biases,
            })
    return in_maps, J_PAD, W_BAND


def _split_excess_waits(nc, max_waits=1):
    """This walrus build rejects instructions carrying multiple semaphore waits
    (Tile's final drain aggregates one per logical proc). Move the excess onto
    preceding same-engine NOPs."""
    from concourse import mybir

    k = 0
    for f in nc.m.functions:
        for blk in f.blocks:
            new_insts = []
            for inst in blk.instructions:
                si = getattr(inst, "sync_info", None)
                if si is not None and si.on_wait and len(si.on_wait) > max_waits:
                    waits = list(si.on_wait)
                    while len(waits) > max_waits:
                        chunk, waits = waits[:max_waits], waits[max_waits:]
                        nop = mybir.InstNoOp(
                            name=f"wsplit-{k}",
                            sync_info=mybir.SyncInfo(on_wait=chunk, on_update=[]),
                            bass_nofuse=True,
                            engine=inst.engine,
                            ins=[], outs=[],
                        )
                        new_insts.append(nop)
                        k += 1
                    si.on_wait = waits
                new_insts.append(inst)
            blk.instructions[:] = new_insts


def _build(J_PAD, W_BAND, split=True):
    import concourse.bass as bass
    import concourse.tile as tile
    from concourse import mybir

    f32 = mybir.dt.float32
    f16 = mybir.dt.float16
    OP = mybir.AluOpType
    ACTF = mybir.ActivationFunctionType

    nc = bass.Bass()
    taps_x = nc.declare_dram_parameter("taps", [4, C, J_PAD], f16, isOutput=False)
    wb_x = nc.declare_dram_parameter("wb", [4, C, J_PAD], f16, isOutput=False)
    qT_x = nc.declare_dram_parameter("qT", [C, HALF], f16, isOutput=False)
    L_x = nc.declare_dram_parameter("L", [HALF, W_BAND], f16, isOutput=False)
    cstarts_x = nc.declare_dram_parameter("cstarts", [1, NT], mybir.dt.int32,
                                          isOutput=False)
    padind_x = nc.declare_dram_parameter("padind", [1, J_PAD], f16, isOutput=False)
    r20fix_x = nc.declare_dram_parameter("r20fix", [128, 1], f32, isOutput=False)
    cntL_x = nc.declare_dram_parameter("cntL", [128, NT], f32, isOutput=False)
    relc_x = nc.declare_dram_parameter("relc", [128, NT], f32, isOutput=False)
    biases_x = nc.declare_dram_parameter("biases", [128, 2 * (NQ + 1)], f32,
                                         isOutput=False)
    loss_x = nc.declare_dram_parameter("loss", [128, NT], f32, isOutput=True)
    ap_x = nc.declare_dram_parameter("ap", [128, NT], f32, isOutput=True)

    NACC = NQ + 1  # 21 accumulator columns per tile

    with tile.TileContext(nc) as tc:
        with (
            tc.tile_pool(name="const", bufs=1) as constp,
            tc.tile_pool(name="dbp", bufs=1) as dbp,
            tc.tile_pool(name="dbscr", bufs=2) as dbscr,
            tc.tile_pool(name="tpool", bufs=2) as tpool,
            tc.tile_pool(name="tlpool", bufs=2) as tlpool,
            tc.tile_pool(name="lpool", bufs=2) as lpool,
            tc.tile_pool(name="scr", bufs=1) as scrp,
            tc.tile_pool(name="acc", bufs=1) as accp,
            tc.tile_pool(name="epi", bufs=1) as epip,
            tc.tile_pool(name="psum", bufs=2, space="PSUM") as psump,
        ):
            # --- constants / inputs resident in SBUF ---
            zeros = constp.tile([128, J_PAD], f16)
            nc.vector.memset(zeros[:], 0.0)
            negfifty = constp.tile([1, 128], f16)
            nc.gpsimd.memset(negfifty[:], -PADSCORE)

            qT = constp.tile([C, HALF], f16)
            nc.sync.dma_start(qT[:], qT_x[:])
            padind = constp.tile([1, J_PAD], f16)
            nc.sync.dma_start(padind[:], padind_x[:])
            biases = constp.tile([128, 2 * (NQ + 1)], f32)
            nc.sync.dma_start(biases[:], biases_x[:])
            r20fix = constp.tile([128, 1], f32)
            nc.sync.dma_start(r20fix[:], r20fix_x[:])
            cntL = constp.tile([128, NT], f32)
            nc.sync.dma_start(cntL[:], cntL_x[:])
            relc = constp.tile([128, NT], f32)
            nc.sync.dma_start(relc[:], relc_x[:])
            cstarts = constp.tile([1, NT], mybir.dt.int32)
            nc.sync.dma_start(cstarts[:], cstarts_x[:])
            cstart_vals = [
                nc.values_load(cstarts[:, T:T + 1],
                               engines=[mybir.EngineType.DVE],
                               min_val=0, max_val=J_PAD - W_BAND,
                               skip_runtime_bounds_check=True)
                for T in range(NT)
            ]

            # --- DB = sum_t taps[t] * wb[t]  (bilinear interpolation) ---
            DB = dbp.tile([C, J_PAD], f16)
            tap_t = [dbscr.tile([C, J_PAD], f16, tag=f"tap{i}", name=f"tap{i}") for i in range(4)]
            wb_t = [dbscr.tile([C, J_PAD], f16, tag=f"wb{i}", name=f"wb{i}") for i in range(4)]
            for t in range(4):
                nc.sync.dma_start(tap_t[t][:], taps_x[t])
                nc.sync.dma_start(wb_t[t][:], wb_x[t])
            prod = dbscr.tile([C, J_PAD], f16, tag="prod")
            nc.vector.tensor_tensor(DB[:], tap_t[0][:], wb_t[0][:], OP.mult)
            for t in range(1, 4):
                nc.vector.tensor_tensor(prod[:], tap_t[t][:], wb_t[t][:], OP.mult)
                nc.vector.tensor_tensor(DB[:], DB[:], prod[:], OP.add)

            # --- accumulators (separate per engine to avoid cross-engine WAW) ---
            R_dve = accp.tile([128, NT * NACC], f32, tag="R_dve")
            R_act = accp.tile([128, NT * NACC], f32, tag="R_act")
            Rb_dve = accp.tile([128, NT * NACC], f32, tag="Rb_dve")
            Rb_act = accp.tile([128, NT * NACC], f32, tag="Rb_act")
            nc.vector.memset(R_dve[:], 0.0)
            nc.scalar.memzero(R_act[:])
            nc.vector.memset(Rb_dve[:], 0.0)
            nc.scalar.memzero(Rb_act[:])

            scr_dve = scrp.tile([128, J_PAD], f16, tag="scr_dve")
            scr_act = scrp.tile([128, J_PAD], f16, tag="scr_act")
            scrb_dve = scrp.tile([128, W_BAND], f16, tag="scrb_dve")
            scrb_act = scrp.tile([128, W_BAND], f16, tag="scrb_act")

            jcs = [(o, min(512, J_PAD - o)) for o in range(0, J_PAD, 512)]

            for T in range(NT):
                ps = psump.tile([128, J_PAD], f32)
                lhs = qT[:, T * 128:(T + 1) * 128]
                for (o, w) in jcs:
                    nc.tensor.matmul(ps[:, o:o + w], lhs, DB[:, o:o + w],
                                     start=True, stop=False)
                    nc.tensor.matmul(ps[:, o:o + w], negfifty[:],
                                     padind[:, o:o + w], start=False, stop=True)

                # t = 9.5 * S   (fp16, SBUF)
                t_t = tpool.tile([128, J_PAD], f16)
                for (o, w) in jcs:
                    nc.scalar.mul(t_t[:, o:o + w], ps[:, o:o + w], A)

                # band slice of t, then tL = (tb + BIG) * L_band
                tb_t = tlpool.tile([128, W_BAND], f16, tag="tb", name="tb")
                nc.vector.tensor_copy(
                    tb_t[:], t_t[:, bass.ds(cstart_vals[T], W_BAND)])
                L_t = lpool.tile([128, W_BAND], f16)
                nc.sync.dma_start(L_t[:], L_x[T * 128:(T + 1) * 128, :])
                tL_t = tlpool.tile([128, W_BAND], f16)
                nc.vector.scalar_tensor_tensor(tL_t[:], tb_t[:], BIG, L_t[:],
                                               OP.add, OP.mult)

                col = T * NACC
                # nbs passes m=1..19
                for m in range(1, NQ):
                    if m in ACT_NBS_MS:
                        nc.scalar.activation(
                            scr_act[:], t_t[:], ACTF.Relu,
                            bias=biases[:, m:m + 1], scale=1.0,
                            accum_out=R_act[:, col + m:col + m + 1])
                    else:
                        nc.vector.scalar_tensor_tensor(
                            scr_dve[:], t_t[:], float(m - A), zeros[:],
                            OP.add, OP.max,
                            accum_out=R_dve[:, col + m:col + m + 1])
                # m=20 linear: sum_j t
                nc.vector.tensor_scalar(
                    scr_dve[:], t_t[:], 1.0, None, OP.mult, OP.add,
                    accum_out=R_dve[:, col + NQ:col + NQ + 1])
                # rec passes m=1..19
                for m in range(1, NQ):
                    if m in ACT_REC_MS:
                        nc.scalar.activation(
                            scrb_act[:], tL_t[:], ACTF.Relu,
                            bias=biases[:, NQ + 1 + m:NQ + 2 + m], scale=1.0,
                            accum_out=Rb_act[:, col + m:col + m + 1])
                    else:
                        nc.vector.scalar_tensor_tensor(
                            scrb_dve[:], tL_t[:], float(m - A - BIG),
                            zeros[:, :W_BAND], OP.add, OP.max,
                            accum_out=Rb_dve[:, col + m:col + m + 1])
                # m=20 linear: sum_j tL
                nc.vector.tensor_scalar(
                    scrb_dve[:], tL_t[:], 1.0, None, OP.mult, OP.add,
                    accum_out=Rb_dve[:, col + NQ:col + NQ + 1])

            # --- epilogue (batched over all tiles) ---
            R = epip.tile([128, NT * NACC], f32, tag="R")
            Rb = epip.tile([128, NT * NACC], f32, tag="Rb")
            nc.vector.tensor_tensor(R[:], R_dve[:], R_act[:], OP.add)
            nc.vector.tensor_tensor(Rb[:], Rb_dve[:], Rb_act[:], OP.add)

            R3 = R.rearrange("p (t m) -> p t m", m=NACC)
            Rb3 = Rb.rearrange("p (t m) -> p t m", m=NACC)

            # R[:, :, 20] += r20fix ; Rb[:, :, 20] += -5.5*cntL
            nc.vector.tensor_scalar(
                R3[:, :, NQ], R3[:, :, NQ], r20fix[:], None, OP.add, OP.bypass)
            nc.vector.scalar_tensor_tensor(
                Rb3[:, :, NQ], cntL[:], -(BIG - A - 1.0), Rb3[:, :, NQ],
                OP.mult, OP.add)

            CN = epip.tile([128, NT, NQ], f32, tag="CN")
            CR = epip.tile([128, NT, NQ], f32, tag="CR")
            REC = epip.tile([128, NT, NQ], f32, tag="REC")
            nc.vector.tensor_tensor(CN[:], R3[:, :, 1:], R3[:, :, :NQ],
                                    OP.subtract)
            nc.vector.tensor_tensor(CR[:], Rb3[:, :, 1:], Rb3[:, :, :NQ],
                                    OP.subtract)
            nc.vector.tensor_copy(REC[:, :, 0:1], CR[:, :, 0:1])
            nc.vector.tensor_tensor(REC[:, :, 1:], CR[:, :, 1:],
                                    CR[:, :, :NQ - 1], OP.subtract)

            rtot = epip.tile([128, NT], f32, tag="rtot")
            nc.vector.tensor_scalar(rtot[:], CR[:, :, NQ - 1], 1e-16, None,
                                    OP.add, OP.bypass)

            # prec = CR / (1e-16 + CN)
            nc.vector.tensor_scalar(CN[:], CN[:], 1e-16, None, OP.add, OP.bypass)
            INV = epip.tile([128, NT, NQ], f32, tag="INV")
            nc.vector.reciprocal(INV[:], CN[:])
            PREC = epip.tile([128, NT, NQ], f32, tag="PREC")
            nc.vector.tensor_tensor(PREC[:], CR[:], INV[:], OP.mult)
            nc.vector.tensor_tensor(PREC[:], PREC[:], REC[:], OP.mult)
            numer = epip.tile([128, NT], f32, tag="numer")
            nc.vector.tensor_reduce(numer[:], PREC[:], mybir.AxisListType.X,
                                    OP.add)

            rinv = epip.tile([128, NT], f32, tag="rinv")
            nc.vector.reciprocal(rinv[:], rtot[:])
            ap = epip.tile([128, NT], f32, tag="ap")
            nc.vector.tensor_tensor(ap[:], numer[:], rinv[:], OP.mult)

            # loss = 0.5 - relc*(ap - 0.5)
            loss = epip.tile([128, NT], f32, tag="loss")
            nc.vector.scalar_tensor_tensor(loss[:], ap[:], -0.5, relc[:],
                                           OP.add, OP.mult)
            nc.vector.tensor_scalar(loss[:], loss[:], -1.0, 0.5,
                                    OP.mult, OP.add)

            nc.sync.dma_start(ap_x[:], ap[:])
            nc.sync.dma_start(loss_x[:], loss[:])

    if split:
        _split_excess_waits(nc)
    return nc


_CACHE = {}


def _get_nc(J_PAD, W_BAND):
    key = (J_PAD, W_BAND)
    if key not in _CACHE:
        _CACHE[key] = _build(J_PAD, W_BAND)
    return _CACHE[key]


def _run(descriptor1, descriptor2, reliability, grid, mask, trace=False):
    from concourse.bass_utils import run_bass_kernel_spmd

    d1 = np.asarray(descriptor1, np.float32)
    d2 = np.asarray(descriptor2, np.float32)
    rel = np.asarray(reliability, np.float32)
    g = np.asarray(grid, np.float32)
    mk = np.asarray(mask)

    in_maps, J_PAD, W_BAND = _host_prep(d1, d2, rel, g, mk)
    nc = _get_nc(J_PAD, W_BAND)
    res = run_bass_kernel_spmd(nc, in_maps, list(range(N_CORES)), trace=trace)

    total = 0.0
    for i in range(N_CORES):
        total += res.results[i]["loss"].astype(np.float64).sum()
    out = np.float32(total / (B * HW))
    return out, res


def kernel(descriptor1, descriptor2, reliability, grid, mask):
    out, _ = _run(descriptor1, descriptor2, reliability, grid, mask)
    return out


# revision 8
# speedup vs baseline: 1.0834x; 1.0834x over previous
"""Quantized AP loss (R2D2 QAPLoss) on 8 Trainium2 NeuronCores.

Sharding: data-parallel over batch (4 images) x query-pixel halves
(2 halves of 2304 pixels) = 8 shards, one per core.

Device algorithm per core (1152 query pixels i, J_PAD masked db columns j):
  DB[c,j]   = sum_t wb[t,c,j] * taps[t,c,j]          (bilinear interp, DVE)
  S[i,j]    = sum_c qT[c,i] * DB[c,j] - 50*pad[j]    (TensorE + rank-1 pad fix)
  t[i,j]    = 9.5 * S[i,j]  (fp16)                   (PSUM->SBUF copy w/ scale)
  tL[i,j]   = (t + 16) * L[i,j]                      (positive-window mask)
  R[m][i]   = sum_j relu(t + m - 9.5)   m=1..19      (relu+accum passes)
  R[20][i]  = sum_j t + r20fix                       (linear shortcut)
  Rb[m][i]  = sum_j relu(tL + m - 25.5) m=1..19
  Rb[20][i] = sum_j tL - 5.5*cntL[i]
  cum_nbs_k = R[k+1]-R[k]; cum_rec_k = Rb[k+1]-Rb[k]     (telescoped clip sums)
  ap_i      = sum_k prec_k*rec_k / rec_total ; loss_i = 0.5 - r_i*(ap_i-0.5)

The identity used: cumsum_{k'<=k} tri_{k'}(s) = clip(9.5*s + k - 8.5, 0, 1)
and clip(x+1,0,1) = relu(x+1) - relu(x), so each quantization bin costs one
relu-accumulate pass, split between the Vector and Scalar engines.
"""
import sys

if "/opt/trn_rl_repo" not in sys.path:
    sys.path.insert(0, "/opt/trn_rl_repo")

import numpy as np

B, C, H, W = 4, 128, 48, 48
HW = H * W
HALF = HW // 2          # 1152 query pixels per core
NT = HALF // 128        # 9 i-tiles per core
NQ = 20
WIN = 4
A = 9.5                 # (NQ-1)/(max-min)
BIG = 16.0              # tL offset (keeps fp16 precision, > max bin bias 10.5)
PADSCORE = 50.0         # rank-1 score pushed onto padded columns
N_CORES = 8

# engine split for the 2*(NQ-1)=38 relu-accum passes (tuned from traces)
ACT_NBS_MS = set(range(1, 11))  # nbs bins handled by ScalarE
ACT_REC_MS = set(range(1, 4))   # rec bins handled by ScalarE


def _host_prep(d1, d2, rel, grid, mask):
    """Build per-core device inputs. Pure indexing / sharding; all FLOP-bearing
    work (interpolation arithmetic, matmuls, binning) happens on device."""
    maskw = mask.reshape(B, HW) == 1
    counts = maskw.sum(1)
    J_PAD = max(128, int(np.ceil(counts.max() / 128) * 128))

    xs = np.arange(HW) // W
    ys = np.arange(HW) % W

    # per (image, i-tile) window of compact columns that can hold positives:
    # rows [xlo-4, xhi+4] of the image; contiguous in compact space.
    cumcnt = np.zeros((B, HW + 1), np.int64)
    for b in range(B):
        cumcnt[b, 1:] = np.cumsum(maskw[b])
    los = np.zeros((B, 2, NT), np.int64)
    his = np.zeros((B, 2, NT), np.int64)
    for b in range(B):
        for h in (0, 1):
            for T in range(NT):
                i0 = h * HALF + T * 128
                i1 = i0 + 127
                xlo = max(i0 // W - WIN, 0)
                xhi = min(i1 // W + WIN, H - 1)
                los[b, h, T] = cumcnt[b, xlo * W]
                his[b, h, T] = cumcnt[b, (xhi + 1) * W]
    wmax = int((his - los).max())
    W_BAND = min(max(128, int(np.ceil((wmax + 2) / 128) * 128)), J_PAD)

    biases = np.zeros((128, 2 * (NQ + 1)), np.float32)
    for m in range(NQ + 1):
        biases[:, m] = m - A
        biases[:, NQ + 1 + m] = m - A - BIG


    rcorrN = np.zeros((128, NT * (NQ + 1)), np.float32)
    rcorrB = np.zeros((128, NT * (NQ + 1)), np.float32)
    for T in range(NT):
        for m in range(1, NQ):
            if m not in ACT_NBS_MS:
                rcorrN[:, T * (NQ + 1) + m] = (m - A) * J_PAD
            if m not in ACT_REC_MS:
                rcorrB[:, T * (NQ + 1) + m] = (m - A - BIG) * W_BAND

    in_maps = []
    for b in range(B):
        g = grid[b]
        gx = (g[..., 0] + 1.0) * W / 2.0 - 0.5
        gy = (g[..., 1] + 1.0) * H / 2.0 - 0.5
        x0 = np.floor(gx)
        y0 = np.floor(gy)
        wx1 = gx - x0
        wx0 = 1.0 - wx1
        wy1 = gy - y0
        wy0 = 1.0 - wy1

        jsel = np.nonzero(maskw[b])[0]
        J_valid = len(jsel)
        n_pad = J_PAD - J_valid
        d2flat = d2[b].reshape(C, HW)

        taps = np.zeros((4, C, J_PAD), np.float16)
        wb = np.zeros((4, C, J_PAD), np.float16)
        for t, (xi, yi, wv) in enumerate(
            ((x0, y0, wx0 * wy0), (x0 + 1, y0, wx1 * wy0),
             (x0, y0 + 1, wx0 * wy1), (x0 + 1, y0 + 1, wx1 * wy1))):
            valid = (xi >= 0) & (xi < W) & (yi >= 0) & (yi < H)
            xc = np.clip(xi, 0, W - 1).astype(np.int64)
            yc = np.clip(yi, 0, H - 1).astype(np.int64)
            f = (yc * W + xc).reshape(HW)[jsel]
            wt = (wv * valid).reshape(HW)[jsel]
            taps[t, :, :J_valid] = d2flat[:, f].astype(np.float16)
            wb[t, :, :J_valid] = wt.astype(np.float16)[None, :]

        padind = np.zeros((1, J_PAD), np.float16)
        padind[0, J_valid:] = 1.0
        r20fix = np.full((128, 1),
                         (A + 1.0) * J_PAD + (PADSCORE * A - (A + 1.0)) * n_pad,
                         np.float32)

        xs_j = xs[jsel]
        ys_j = ys[jsel]
        for h in (0, 1):
            irange = np.arange(h * HALF, (h + 1) * HALF)
            L = ((np.abs(xs[irange][:, None] - xs_j[None, :]) <= WIN)
                 & (np.abs(ys[irange][:, None] - ys_j[None, :]) <= WIN))
            Lp = np.zeros((HALF, J_PAD), np.float16)
            Lp[:, :J_valid] = L
            # band-aligned positive mask: L_band[T*128+p, :] =
            #   Lp[T*128+p, cstart[T]:cstart[T]+W_BAND]
            cstarts = np.zeros((1, NT), np.int32)
            L_band = np.zeros((HALF, W_BAND), np.float16)
            for T in range(NT):
                cs = int(min(max(los[b, h, T] - 1, 0), J_PAD - W_BAND))
                cs &= ~1  # even start keeps fp16 slices 4B-aligned
                cstarts[0, T] = cs
                L_band[T * 128:(T + 1) * 128] = Lp[T * 128:(T + 1) * 128,
                                                   cs:cs + W_BAND]
            assert int((L_band.sum())) == int(Lp.sum())
            cntL = np.ascontiguousarray(
                L.sum(1).astype(np.float32).reshape(NT, 128).T)
            relc = np.ascontiguousarray(
                rel[b, 0].reshape(HW)[irange].astype(np.float32)
                .reshape(NT, 128).T)
            qT = np.ascontiguousarray(
                d1[b].reshape(C, HW)[:, irange].astype(np.float16))
            in_maps.append({
                "taps": taps, "wb": wb, "qT": qT, "L": L_band,
                "cstarts": cstarts, "rcorrN": rcorrN, "rcorrB": rcorrB,
                "padind": padind, "r20fix": r20fix, "cntL": cntL,
                "relc": relc, "biases": biases,
            })
    return in_maps, J_PAD, W_BAND


def _split_excess_waits(nc, max_waits=1):
    """This walrus build rejects instructions carrying multiple semaphore waits
    (Tile's final drain aggregates one per logical proc). Move the excess onto
    preceding same-engine NOPs."""
    from concourse import mybir

    k = 0
    for f in nc.m.functions:
        for blk in f.blocks:
            new_insts = []
            for inst in blk.instructions:
                si = getattr(inst, "sync_info", None)
                if si is not None and si.on_wait and len(si.on_wait) > max_waits:
                    waits = list(si.on_wait)
                    while len(waits) > max_waits:
                        chunk, waits = waits[:max_waits], waits[max_waits:]
                        nop = mybir.InstNoOp(
                            name=f"wsplit-{k}",
                            sync_info=mybir.SyncInfo(on_wait=chunk, on_update=[]),
                            bass_nofuse=True,
                            engine=inst.engine,
                            ins=[], outs=[],
                        )
                        new_insts.append(nop)
                        k += 1
                    si.on_wait = waits
                new_insts.append(inst)
            blk.instructions[:] = new_insts


def _build(J_PAD, W_BAND, split=True):
    import concourse.bass as bass
    import concourse.tile as tile
    from concourse import mybir

    f32 = mybir.dt.float32
    f16 = mybir.dt.float16
    OP = mybir.AluOpType
    ACTF = mybir.ActivationFunctionType

    nc = bass.Bass()
    taps_x = nc.declare_dram_parameter("taps", [4, C, J_PAD], f16, isOutput=False)
    wb_x = nc.declare_dram_parameter("wb", [4, C, J_PAD], f16, isOutput=False)
    qT_x = nc.declare_dram_parameter("qT", [C, HALF], f16, isOutput=False)
    L_x = nc.declare_dram_parameter("L", [HALF, W_BAND], f16, isOutput=False)
    cstarts_x = nc.declare_dram_parameter("cstarts", [1, NT], mybir.dt.int32,
                                          isOutput=False)
    padind_x = nc.declare_dram_parameter("padind", [1, J_PAD], f16, isOutput=False)
    r20fix_x = nc.declare_dram_parameter("r20fix", [128, 1], f32, isOutput=False)
    cntL_x = nc.declare_dram_parameter("cntL", [128, NT], f32, isOutput=False)
    relc_x = nc.declare_dram_parameter("relc", [128, NT], f32, isOutput=False)
    biases_x = nc.declare_dram_parameter("biases", [128, 2 * (NQ + 1)], f32,
                                         isOutput=False)
    rcorrN_x = nc.declare_dram_parameter("rcorrN", [128, NT * (NQ + 1)], f32,
                                         isOutput=False)
    rcorrB_x = nc.declare_dram_parameter("rcorrB", [128, NT * (NQ + 1)], f32,
                                         isOutput=False)
    loss_x = nc.declare_dram_parameter("loss", [128, NT], f32, isOutput=True)
    ap_x = nc.declare_dram_parameter("ap", [128, NT], f32, isOutput=True)

    NACC = NQ + 1  # 21 accumulator columns per tile

    with tile.TileContext(nc) as tc:
        with (
            tc.tile_pool(name="const", bufs=1) as constp,
            tc.tile_pool(name="dbp", bufs=1) as dbp,
            tc.tile_pool(name="dbscr", bufs=2) as dbscr,
            tc.tile_pool(name="tpool", bufs=2) as tpool,
            tc.tile_pool(name="tlpool", bufs=2) as tlpool,
            tc.tile_pool(name="lpool", bufs=2) as lpool,
            tc.tile_pool(name="scr", bufs=1) as scrp,
            tc.tile_pool(name="acc", bufs=1) as accp,
            tc.tile_pool(name="epi", bufs=1) as epip,
            tc.tile_pool(name="psum", bufs=2, space="PSUM") as psump,
        ):
            # --- constants / inputs resident in SBUF ---
            negfifty = constp.tile([1, 128], f16)
            nc.gpsimd.memset(negfifty[:], -PADSCORE)

            qT = constp.tile([C, HALF], f16)
            nc.sync.dma_start(qT[:], qT_x[:])
            padind = constp.tile([1, J_PAD], f16)
            nc.sync.dma_start(padind[:], padind_x[:])
            biases = constp.tile([128, 2 * (NQ + 1)], f32)
            nc.sync.dma_start(biases[:], biases_x[:])
            r20fix = constp.tile([128, 1], f32)
            nc.sync.dma_start(r20fix[:], r20fix_x[:])
            cntL = constp.tile([128, NT], f32)
            nc.sync.dma_start(cntL[:], cntL_x[:])
            relc = constp.tile([128, NT], f32)
            nc.sync.dma_start(relc[:], relc_x[:])
            cstarts = constp.tile([1, NT], mybir.dt.int32)
            nc.sync.dma_start(cstarts[:], cstarts_x[:])
            cstart_vals = [
                nc.values_load(cstarts[:, T:T + 1],
                               engines=[mybir.EngineType.DVE],
                               min_val=0, max_val=J_PAD - W_BAND,
                               skip_runtime_bounds_check=True)
                for T in range(NT)
            ]

            # --- DB = sum_t taps[t] * wb[t]  (bilinear interpolation) ---
            DB = dbp.tile([C, J_PAD], f16)
            tap_t = [dbscr.tile([C, J_PAD], f16, tag=f"tap{i}", name=f"tap{i}") for i in range(4)]
            wb_t = [dbscr.tile([C, J_PAD], f16, tag=f"wb{i}", name=f"wb{i}") for i in range(4)]
            for t in range(4):
                nc.sync.dma_start(tap_t[t][:], taps_x[t])
                nc.sync.dma_start(wb_t[t][:], wb_x[t])
            prod = dbscr.tile([C, J_PAD], f16, tag="prod")
            nc.vector.tensor_tensor(DB[:], tap_t[0][:], wb_t[0][:], OP.mult)
            for t in range(1, 4):
                nc.vector.tensor_tensor(prod[:], tap_t[t][:], wb_t[t][:], OP.mult)
                nc.vector.tensor_tensor(DB[:], DB[:], prod[:], OP.add)

            # --- accumulators (separate per engine to avoid cross-engine WAW) ---
            R_dve = accp.tile([128, NT * NACC], f32, tag="R_dve")
            R_act = accp.tile([128, NT * NACC], f32, tag="R_act")
            Rb_dve = accp.tile([128, NT * NACC], f32, tag="Rb_dve")
            Rb_act = accp.tile([128, NT * NACC], f32, tag="Rb_act")
            nc.vector.memset(R_dve[:], 0.0)
            nc.vector.memset(Rb_dve[:], 0.0)
            nc.sync.dma_start(R_act[:], rcorrN_x[:])
            nc.sync.dma_start(Rb_act[:], rcorrB_x[:])

            scr_dve = scrp.tile([128, J_PAD], f16, tag="scr_dve")
            scr_act = scrp.tile([128, J_PAD], f16, tag="scr_act")
            scrb_dve = scrp.tile([128, W_BAND], f16, tag="scrb_dve")
            scrb_act = scrp.tile([128, W_BAND], f16, tag="scrb_act")

            jcs = [(o, min(512, J_PAD - o)) for o in range(0, J_PAD, 512)]

            for T in range(NT):
                ps = psump.tile([128, J_PAD], f32)
                lhs = qT[:, T * 128:(T + 1) * 128]
                for (o, w) in jcs:
                    nc.tensor.matmul(ps[:, o:o + w], lhs, DB[:, o:o + w],
                                     start=True, stop=False)
                    nc.tensor.matmul(ps[:, o:o + w], negfifty[:],
                                     padind[:, o:o + w], start=False, stop=True)

                # t = 9.5 * S   (fp16, SBUF)
                t_t = tpool.tile([128, J_PAD], f16)
                for (o, w) in jcs:
                    nc.scalar.mul(t_t[:, o:o + w], ps[:, o:o + w], A)

                # band slice of t, then tL = (tb + BIG) * L_band
                tb_t = tlpool.tile([128, W_BAND], f16, tag="tb", name="tb")
                nc.vector.tensor_copy(
                    tb_t[:], t_t[:, bass.ds(cstart_vals[T], W_BAND)])
                L_t = lpool.tile([128, W_BAND], f16)
                nc.sync.dma_start(L_t[:], L_x[T * 128:(T + 1) * 128, :])
                tL_t = tlpool.tile([128, W_BAND], f16)
                nc.vector.scalar_tensor_tensor(tL_t[:], tb_t[:], BIG, L_t[:],
                                               OP.add, OP.mult)

                col = T * NACC
                # nbs passes m=1..19
                for m in range(1, NQ):
                    if m in ACT_NBS_MS:
                        nc.scalar.activation(
                            scr_act[:], t_t[:], ACTF.Relu,
                            bias=biases[:, m:m + 1], scale=1.0,
                            accum_out=R_act[:, col + m:col + m + 1])
                    else:
                        nc.vector.tensor_scalar(
                            scr_dve[:], t_t[:], float(A - m), None,
                            OP.max, OP.add,
                            accum_out=R_dve[:, col + m:col + m + 1])
                # m=20 linear: sum_j t
                nc.vector.tensor_scalar(
                    scr_dve[:], t_t[:], 1.0, None, OP.mult, OP.add,
                    accum_out=R_dve[:, col + NQ:col + NQ + 1])
                # rec passes m=1..19
                for m in range(1, NQ):
                    if m in ACT_REC_MS:
                        nc.scalar.activation(
                            scrb_act[:], tL_t[:], ACTF.Relu,
                            bias=biases[:, NQ + 1 + m:NQ + 2 + m], scale=1.0,
                            accum_out=Rb_act[:, col + m:col + m + 1])
                    else:
                        nc.vector.tensor_scalar(
                            scrb_dve[:], tL_t[:], float(A + BIG - m), None,
                            OP.max, OP.add,
                            accum_out=Rb_dve[:, col + m:col + m + 1])
                # m=20 linear: sum_j tL
                nc.vector.tensor_scalar(
                    scrb_dve[:], tL_t[:], 1.0, None, OP.mult, OP.add,
                    accum_out=Rb_dve[:, col + NQ:col + NQ + 1])

            # --- epilogue (batched over all tiles) ---
            R = epip.tile([128, NT * NACC], f32, tag="R")
            Rb = epip.tile([128, NT * NACC], f32, tag="Rb")
            nc.vector.tensor_tensor(R[:], R_dve[:], R_act[:], OP.add)
            nc.vector.tensor_tensor(Rb[:], Rb_dve[:], Rb_act[:], OP.add)

            R3 = R.rearrange("p (t m) -> p t m", m=NACC)
            Rb3 = Rb.rearrange("p (t m) -> p t m", m=NACC)

            # R[:, :, 20] += r20fix ; Rb[:, :, 20] += -5.5*cntL
            nc.vector.tensor_scalar(
                R3[:, :, NQ], R3[:, :, NQ], r20fix[:], None, OP.add, OP.bypass)
            nc.vector.scalar_tensor_tensor(
                Rb3[:, :, NQ], cntL[:], -(BIG - A - 1.0), Rb3[:, :, NQ],
                OP.mult, OP.add)

            CN = epip.tile([128, NT, NQ], f32, tag="CN")
            CR = epip.tile([128, NT, NQ], f32, tag="CR")
            REC = epip.tile([128, NT, NQ], f32, tag="REC")
            nc.vector.tensor_tensor(CN[:], R3[:, :, 1:], R3[:, :, :NQ],
                                    OP.subtract)
            nc.vector.tensor_tensor(CR[:], Rb3[:, :, 1:], Rb3[:, :, :NQ],
                                    OP.subtract)
            nc.vector.tensor_copy(REC[:, :, 0:1], CR[:, :, 0:1])
            nc.vector.tensor_tensor(REC[:, :, 1:], CR[:, :, 1:],
                                    CR[:, :, :NQ - 1], OP.subtract)

            rtot = epip.tile([128, NT], f32, tag="rtot")
            nc.vector.tensor_scalar(rtot[:], CR[:, :, NQ - 1], 1e-16, None,
                                    OP.add, OP.bypass)

            # prec = CR / (1e-16 + CN)
            nc.vector.tensor_scalar(CN[:], CN[:], 1e-16, None, OP.add, OP.bypass)
            INV = epip.tile([128, NT, NQ], f32, tag="INV")
            nc.vector.reciprocal(INV[:], CN[:])
            PREC = epip.tile([128, NT, NQ], f32, tag="PREC")
            nc.vector.tensor_tensor(PREC[:], CR[:], INV[:], OP.mult)
            nc.vector.tensor_tensor(PREC[:], PREC[:], REC[:], OP.mult)
            numer = epip.tile([128, NT], f32, tag="numer")
            nc.vector.tensor_reduce(numer[:], PREC[:], mybir.AxisListType.X,
                                    OP.add)

            rinv = epip.tile([128, NT], f32, tag="rinv")
            nc.vector.reciprocal(rinv[:], rtot[:])
            ap = epip.tile([128, NT], f32, tag="ap")
            nc.vector.tensor_tensor(ap[:], numer[:], rinv[:], OP.mult)

            # loss = 0.5 - relc*(ap - 0.5)
            loss = epip.tile([128, NT], f32, tag="loss")
            nc.vector.scalar_tensor_tensor(loss[:], ap[:], -0.5, relc[:],
                                           OP.add, OP.mult)
            nc.vector.tensor_scalar(loss[:], loss[:], -1.0, 0.5,
                                    OP.mult, OP.add)

            nc.sync.dma_start(ap_x[:], ap[:])
            nc.sync.dma_start(loss_x[:], loss[:])

    if split:
        _split_excess_waits(nc)
    return nc


_CACHE = {}


def _get_nc(J_PAD, W_BAND):
    key = (J_PAD, W_BAND)
    if key not in _CACHE:
        _CACHE[key] = _build(J_PAD, W_BAND)
    return _CACHE[key]


def _run(descriptor1, descriptor2, reliability, grid, mask, trace=False):
    from concourse.bass_utils import run_bass_kernel_spmd

    d1 = np.asarray(descriptor1, np.float32)
    d2 = np.asarray(descriptor2, np.float32)
    rel = np.asarray(reliability, np.float32)
    g = np.asarray(grid, np.float32)
    mk = np.asarray(mask)

    in_maps, J_PAD, W_BAND = _host_prep(d1, d2, rel, g, mk)
    nc = _get_nc(J_PAD, W_BAND)
    res = run_bass_kernel_spmd(nc, in_maps, list(range(N_CORES)), trace=trace)

    total = 0.0
    for i in range(N_CORES):
        total += res.results[i]["loss"].astype(np.float64).sum()
    out = np.float32(total / (B * HW))
    return out, res


def kernel(descriptor1, descriptor2, reliability, grid, mask):
    out, _ = _run(descriptor1, descriptor2, reliability, grid, mask)
    return out
